# revision 2
# baseline (speedup 1.0000x reference)
"""MinGRU forward on 8 Trainium2 NeuronCores.

Reference computation (per batch b):
    k       = x @ Wz + bz                 # [T, H]
    z       = sigmoid(k)
    c       = 1 - z = sigmoid(-k)
    htilde  = g(x @ Wh + bh)              # g(a) = a+0.5 if a>=0 else sigmoid(a)
                                          #      = max(a+0.5, sigmoid(a))
    h[0]    = g(h_0)
    h[t]    = c[t-1]*h[t-1] + z[t-1]*htilde[t-1]   (t = 1..T)
    out     = h                           # [T+1, H]

The log-space cumlogsumexp in the reference is exactly this linear
recurrence (all quantities positive, coefficients in (0,1), so the
linear form is numerically stable).

Sharding: data-parallel over batch, one batch per core, weights
replicated.

Device layout: matmuls run with H on the PSUM partition dim and T on
the free dim — the layout tensor_tensor_scan needs to run the
recurrence along T at vector speed. x is transposed AND cast to fp16
on the host, so the device only does plain (fast, parallel) DMA loads
— no DMA-transpose, which would serialize against every other DMA via
Tile's deadlock guard. The device writes the output transposed
([H, T+1]); the host transposes during the unshard.

Schedule notes (from perfetto trace analysis of the previous version):
  - ~36 junk matmuls at kernel start warm the PE HAM clock gate
    (1.2 -> 2.4 GHz) while the first weight/x DMAs are in flight.
  - Head DMA order wz0, xt0, wh0, wz1, wh1, ... lets the PE start
    ~7us earlier; chunk 0 consumes weight k-slices in arrival order
    (k-outer over m 0-3, then m-outer for m 4-7).
  - The last two chunks are 256 wide to shrink the post-matmul
    ACT->DVE scan tail, and run pa (htilde path) before pk so the
    gate chain overlaps the remaining matmuls.
"""

import numpy as np

B, T, D, H = 8, 4096, 1024, 1024
P = 128
TCH = 512                 # main time-chunk (one PSUM bank of fp32)
KO = D // P               # contraction tiles
MO = H // P               # output-channel tiles
CHUNKS = [512] * 7 + [256, 256]   # sum = 4096
NWARM = 36                # HAM warm-up matmuls

_PROGRAM_CACHE = {}


def _build_program():
    import concourse.bacc as bacc
    import concourse.mybir as mybir
    import concourse.tile as tile

    fp32 = mybir.dt.float32
    fp16 = mybir.dt.float16
    SIG = mybir.ActivationFunctionType.Sigmoid
    MUL = mybir.AluOpType.mult
    ADD = mybir.AluOpType.add
    MAX = mybir.AluOpType.max

    nc = bacc.Bacc("TRN2", target_bir_lowering=False)

    # x arrives pre-transposed from the host: [D, T]
    xt_ext = nc.declare_dram_parameter("x", [D, T], fp16, isOutput=False)
    h0_ext = nc.declare_dram_parameter("h_0", [H], fp32, isOutput=False)
    wz_ext = nc.declare_dram_parameter("Wz", [D, H], fp16, isOutput=False)
    bz_ext = nc.declare_dram_parameter("bz", [H], fp32, isOutput=False)
    wh_ext = nc.declare_dram_parameter("Wh", [D, H], fp16, isOutput=False)
    bh_ext = nc.declare_dram_parameter("bh", [H], fp32, isOutput=False)
    # transposed output; host untransposes during the gather
    out_ext = nc.declare_dram_parameter("out", [H, T + 1], fp32, isOutput=True)

    xt_r = xt_ext.rearrange("(ko ki) t -> ki ko t", ki=P)
    wz_r = wz_ext.rearrange("(ko ki) h -> ki ko h", ki=P)
    wh_r = wh_ext.rearrange("(ko ki) h -> ki ko h", ki=P)

    chunk_t0 = []
    t0 = 0
    for w in CHUNKS:
        chunk_t0.append(t0)
        t0 += w
    NCH = len(CHUNKS)

    with tile.TileContext(nc) as tc:
        with (
            tc.tile_pool(name="const", bufs=1) as const_pool,
            tc.tile_pool(name="w", bufs=1) as w_pool,
            tc.tile_pool(name="xt", bufs=3) as xt_pool,
            tc.tile_pool(name="ht", bufs=2) as ht_pool,
            tc.tile_pool(name="gate", bufs=3) as gate_pool,
            tc.tile_pool(name="psp", bufs=4, space="PSUM") as psum_p,
        ):
            # --- PE warm-up: junk matmuls so the HAM clock gate is at
            # 2.4 GHz by the time the first real matmul's operands land.
            junk = const_pool.tile([P, P], fp16)
            nc.vector.memset(junk, 0.0)
            warm_ps = psum_p.tile([P, P], fp32, tag="pk", name="warm")
            for _ in range(NWARM):
                nc.tensor.matmul(warm_ps, junk, junk, start=True, stop=True)

            # --- Head DMAs on the SP ring, in the exact order chunk 0
            # consumes them: wz ko=0, x chunk 0, wh ko=0, then the
            # remaining weight k-slices (256 KB each).
            wz_sb = w_pool.tile([P, KO, H], fp16)
            wh_sb = w_pool.tile([P, KO, H], fp16)

            xt_tiles = []

            def issue_xt(ci):
                tch = CHUNKS[ci]
                c0 = chunk_t0[ci]
                xt_sb = xt_pool.tile([P, KO, TCH], fp16, tag="xt", name="xt")
                nc.sync.dma_start(xt_sb[:, :, :tch], xt_r[:, :, c0:c0 + tch])
                xt_tiles.append(xt_sb)

            nc.sync.dma_start(wz_sb[:, 0], wz_r[:, 0])
            issue_xt(0)
            nc.sync.dma_start(wh_sb[:, 0], wh_r[:, 0])
            for ko in range(1, KO):
                nc.sync.dma_start(wz_sb[:, ko], wz_r[:, ko])
                nc.sync.dma_start(wh_sb[:, ko], wh_r[:, ko])
            issue_xt(1)

            # --- Small constants on the ACT ring (parallel with the SP
            # ring; nothing here is on the PE critical path).
            bz_sb = const_pool.tile([P, MO], fp32)
            nc.scalar.dma_start(bz_sb, bz_ext.rearrange("(mo mi) -> mi mo", mi=P))
            bh_sb = const_pool.tile([P, MO], fp32)
            nc.scalar.dma_start(bh_sb, bh_ext.rearrange("(mo mi) -> mi mo", mi=P))
            h0_sb = const_pool.tile([P, MO], fp32)
            nc.scalar.dma_start(h0_sb, h0_ext.rearrange("(mo mi) -> mi mo", mi=P))

            nbz_sb = const_pool.tile([P, MO], fp32)
            nc.vector.tensor_scalar_mul(nbz_sb, bz_sb, -1.0)
            bhp5_sb = const_pool.tile([P, MO], fp32)
            nc.vector.tensor_scalar_add(bhp5_sb, bh_sb, 0.5)

            # h[0] = g(h_0) = max(h_0 + 0.5, sigmoid(h_0))
            s0_sb = const_pool.tile([P, MO], fp32)
            nc.scalar.activation(s0_sb, h0_sb, SIG)
            gh0_sb = const_pool.tile([P, MO], fp32)
            nc.vector.scalar_tensor_tensor(gh0_sb, h0_sb, 0.5, s0_sb, op0=ADD, op1=MAX)
            nc.scalar.dma_start(
                out_ext[:, 0].rearrange("(mo mi) -> mi mo", mi=P), gh0_sb
            )

            prev_ht = None  # previous chunk's scan output (carries the state)
            prev_tch = TCH

            def gates_scan_store(m, c0, tch, pk, pa, ht_sb, tail=False):
                # pa completes before pk (pa-first matmul order), so s/g
                # overlap the pk matmuls; z -> c -> v -> scan is the tail
                # chain after pk.
                s_sb = gate_pool.tile([P, TCH], fp32, tag="s", name="s")[:, :tch]
                nc.scalar.activation(s_sb, pa, SIG, bias=bh_sb[:, m:m + 1])
                z_sb = gate_pool.tile([P, TCH], fp32, tag="z", name="z")[:, :tch]
                nc.scalar.activation(z_sb, pk, SIG, bias=bz_sb[:, m:m + 1])
                c_sb = gate_pool.tile([P, TCH], fp32, tag="c", name="c")[:, :tch]
                nc.scalar.activation(
                    c_sb, pk, SIG, bias=nbz_sb[:, m:m + 1], scale=-1.0
                )
                g_sb = gate_pool.tile([P, TCH], fp32, tag="g", name="g")[:, :tch]
                nc.vector.scalar_tensor_tensor(
                    g_sb, pa, bhp5_sb[:, m:m + 1], s_sb, op0=ADD, op1=MAX
                )
                v_sb = gate_pool.tile([P, TCH], fp32, tag="v", name="v")[:, :tch]
                # steady state: gpsimd (throughput, keeps DVE free for the
                # scan); kernel tail: DVE (latency)
                if tail:
                    nc.vector.tensor_mul(v_sb, z_sb, g_sb)
                else:
                    nc.gpsimd.tensor_mul(v_sb, z_sb, g_sb)

                init = (
                    gh0_sb[:, m:m + 1]
                    if prev_ht is None
                    else prev_ht[:, m, prev_tch - 1:prev_tch]
                )
                nc.vector.tensor_tensor_scan(
                    ht_sb[:, m, :tch], c_sb, v_sb, init, op0=MUL, op1=ADD
                )
                nc.sync.dma_start(
                    out_ext[m * P:(m + 1) * P, 1 + c0:1 + c0 + tch],
                    ht_sb[:, m, :tch],
                )

            for ci in range(NCH):
                tch = CHUNKS[ci]
                c0 = chunk_t0[ci]
                if ci + 2 < NCH:
                    issue_xt(ci + 2)
                xt_sb = xt_tiles[ci]
                ht_sb = ht_pool.tile([P, MO, TCH], fp32)

                if ci == 0:
                    # Pass A (m 0-3): k-outer so matmuls consume the
                    # weight k-slices in DMA arrival order — the PE
                    # starts as soon as wz0 + x0 land and never stalls
                    # on the weight stream.
                    GQ = 4
                    pks = [
                        psum_p.tile([P, TCH], fp32, tag="pk", name="pk")
                        for _ in range(GQ)
                    ]
                    pas = [
                        psum_p.tile([P, TCH], fp32, tag="pa", name="pa")
                        for _ in range(GQ)
                    ]
                    for ko in range(KO):
                        for q in range(GQ):
                            nc.tensor.matmul(
                                pks[q],
                                wz_sb[:, ko, q * P:(q + 1) * P],
                                xt_sb[:, ko, :tch],
                                start=(ko == 0),
                                stop=(ko == KO - 1),
                            )
                        for q in range(GQ):
                            nc.tensor.matmul(
                                pas[q],
                                wh_sb[:, ko, q * P:(q + 1) * P],
                                xt_sb[:, ko, :tch],
                                start=(ko == 0),
                                stop=(ko == KO - 1),
                            )
                    for q in range(GQ):
                        gates_scan_store(q, c0, tch, pks[q], pas[q], ht_sb)
                    m_range = range(GQ, MO)
                else:
                    m_range = range(MO)

                for m in m_range:
                    pa = psum_p.tile([P, TCH], fp32, tag="pa", name="pa")[:, :tch]
                    pk = psum_p.tile([P, TCH], fp32, tag="pk", name="pk")[:, :tch]
                    for ko in range(KO):
                        nc.tensor.matmul(
                            pa,
                            wh_sb[:, ko, m * P:(m + 1) * P],
                            xt_sb[:, ko, :tch],
                            start=(ko == 0),
                            stop=(ko == KO - 1),
                        )
                    for ko in range(KO):
                        nc.tensor.matmul(
                            pk,
                            wz_sb[:, ko, m * P:(m + 1) * P],
                            xt_sb[:, ko, :tch],
                            start=(ko == 0),
                            stop=(ko == KO - 1),
                        )
                    tail = ci == NCH - 1 and m >= MO - 2
                    gates_scan_store(m, c0, tch, pk, pa, ht_sb, tail=tail)

                prev_ht = ht_sb
                prev_tch = tch

    nc.finalize()
    return nc


def _get_program():
    if "v3" not in _PROGRAM_CACHE:
        _PROGRAM_CACHE["v3"] = _build_program()
    return _PROGRAM_CACHE["v3"]


def run(x, h_0, Wz, bz, Wh, bh, trace=False):
    from concourse.bass_utils import run_bass_kernel_spmd

    nc = _get_program()
    wz16 = np.ascontiguousarray(np.asarray(Wz, dtype=np.float16))
    wh16 = np.ascontiguousarray(np.asarray(Wh, dtype=np.float16))
    bz32 = np.ascontiguousarray(np.asarray(bz, dtype=np.float32))
    bh32 = np.ascontiguousarray(np.asarray(bh, dtype=np.float32))
    in_maps = [
        {
            "x": np.ascontiguousarray(np.asarray(x[b], dtype=np.float16).T),
            "h_0": np.ascontiguousarray(
                np.asarray(h_0[b], dtype=np.float32).reshape(H)
            ),
            "Wz": wz16,
            "bz": bz32,
            "Wh": wh16,
            "bh": bh32,
        }
        for b in range(B)
    ]
    res = run_bass_kernel_spmd(nc, in_maps, list(range(B)), trace=trace)
    out = np.stack(
        [np.ascontiguousarray(res.results[b]["out"].T) for b in range(B)], axis=0
    )
    return out, res


def kernel(x, h_0, Wz, bz, Wh, bh):
    out, _ = run(x, h_0, Wz, bz, Wh, bh)
    return out


# revision 6
# speedup vs baseline: 1.0510x; 1.0510x over previous
"""MinGRU forward on 8 Trainium2 NeuronCores.

Reference computation (per batch b):
    k       = x @ Wz + bz                 # [T, H]
    z       = sigmoid(k)
    c       = 1 - z = sigmoid(-k)
    htilde  = g(x @ Wh + bh)              # g(a) = a+0.5 if a>=0 else sigmoid(a)
                                          #      = max(a+0.5, sigmoid(a))
    h[0]    = g(h_0)
    h[t]    = c[t-1]*h[t-1] + z[t-1]*htilde[t-1]   (t = 1..T)
    out     = h                           # [T+1, H]

The log-space cumlogsumexp in the reference is exactly this linear
recurrence (all quantities positive, coefficients in (0,1), so the
linear form is numerically stable).

Sharding: data-parallel over batch, one batch per core, weights
replicated.

Device layout: matmuls run with H on the PSUM partition dim and T on
the free dim — the layout tensor_tensor_scan needs to run the
recurrence along T at vector speed. x is transposed AND cast to fp16
on the host, so the device only does plain (fast, parallel) DMA loads
— no DMA-transpose, which would serialize against every other DMA via
Tile's deadlock guard. The device writes the output transposed
([H, T+1]); the host transposes during the unshard.

Schedule notes (from perfetto trace analysis of the previous version):
  - ~36 junk matmuls at kernel start warm the PE HAM clock gate
    (1.2 -> 2.4 GHz) while the first weight/x DMAs are in flight.
  - Head DMA order wz0, xt0, wh0, wz1, wh1, ... lets the PE start
    ~7us earlier; chunk 0 consumes weight k-slices in arrival order
    (k-outer over m 0-3, then m-outer for m 4-7).
  - The last two chunks are 256 wide to shrink the post-matmul
    ACT->DVE scan tail, and run pa (htilde path) before pk so the
    gate chain overlaps the remaining matmuls.
"""

import numpy as np

B, T, D, H = 8, 4096, 1024, 1024
P = 128
TCH = 512                 # main time-chunk (one PSUM bank of fp32)
KO = D // P               # contraction tiles
MO = H // P               # output-channel tiles
CHUNKS = [512] * 7 + [256, 256]   # sum = 4096
NWARM = 40                # HAM warm-up matmuls
GQ = 3                    # chunk-0 pass-A group width (m 0..GQ-1)

_PROGRAM_CACHE = {}


def _build_program():
    import concourse.bacc as bacc
    import concourse.mybir as mybir
    import concourse.tile as tile

    fp32 = mybir.dt.float32
    fp16 = mybir.dt.float16
    SIG = mybir.ActivationFunctionType.Sigmoid
    MUL = mybir.AluOpType.mult
    ADD = mybir.AluOpType.add
    MAX = mybir.AluOpType.max

    nc = bacc.Bacc("TRN2", target_bir_lowering=False)

    # x arrives pre-transposed from the host: [D, T]
    xt_ext = nc.declare_dram_parameter("x", [D, T], fp16, isOutput=False)
    h0_ext = nc.declare_dram_parameter("h_0", [H], fp32, isOutput=False)
    wz_ext = nc.declare_dram_parameter("Wz", [D, H], fp16, isOutput=False)
    bz_ext = nc.declare_dram_parameter("bz", [H], fp32, isOutput=False)
    wh_ext = nc.declare_dram_parameter("Wh", [D, H], fp16, isOutput=False)
    bh_ext = nc.declare_dram_parameter("bh", [H], fp32, isOutput=False)
    # transposed output; host untransposes during the gather
    out_ext = nc.declare_dram_parameter("out", [H, T + 1], fp32, isOutput=True)

    xt_r = xt_ext.rearrange("(ko ki) t -> ki ko t", ki=P)
    wz_r = wz_ext.rearrange("(ko ki) h -> ki ko h", ki=P)
    wh_r = wh_ext.rearrange("(ko ki) h -> ki ko h", ki=P)

    chunk_t0 = []
    t0 = 0
    for w in CHUNKS:
        chunk_t0.append(t0)
        t0 += w
    NCH = len(CHUNKS)

    with tile.TileContext(nc) as tc:
        with (
            tc.tile_pool(name="const", bufs=1) as const_pool,
            tc.tile_pool(name="w", bufs=1) as w_pool,
            tc.tile_pool(name="xt", bufs=3) as xt_pool,
            tc.tile_pool(name="ht", bufs=2) as ht_pool,
            tc.tile_pool(name="gate", bufs=3) as gate_pool,
            tc.tile_pool(name="psp", bufs=4, space="PSUM") as psum_p,
        ):
            # --- PE warm-up: junk matmuls so the HAM clock gate is at
            # 2.4 GHz by the time the first real matmul's operands land.
            junk = const_pool.tile([P, P], fp16)
            nc.vector.memset(junk, 0.0)
            warm_ps = psum_p.tile([P, P], fp32, tag="pk", name="warm")
            for _ in range(NWARM):
                nc.tensor.matmul(warm_ps, junk, junk, start=True, stop=True)

            # --- Head DMAs on the SP ring, in the exact order chunk 0
            # consumes them: wz ko=0, x chunk 0, wh ko=0, then the
            # remaining weight k-slices (256 KB each).
            wz_sb = w_pool.tile([P, KO, H], fp16)
            wh_sb = w_pool.tile([P, KO, H], fp16)

            xt_tiles = []

            def issue_xt(ci):
                tch = CHUNKS[ci]
                c0 = chunk_t0[ci]
                xt_sb = xt_pool.tile([P, KO, TCH], fp16, tag="xt", name="xt")
                nc.sync.dma_start(xt_sb[:, :, :tch], xt_r[:, :, c0:c0 + tch])
                xt_tiles.append(xt_sb)

            # chunk 0's x in two halves so the first matmul (ko=0) only
            # waits for 512 KB, not 1 MB (subtile deps scope the wait)
            nc.sync.dma_start(wz_sb[:, 0], wz_r[:, 0])
            xt0_sb = xt_pool.tile([P, KO, TCH], fp16, tag="xt", name="xt")
            nc.sync.dma_start(xt0_sb[:, :KO // 2], xt_r[:, :KO // 2, 0:TCH])
            nc.sync.dma_start(xt0_sb[:, KO // 2:], xt_r[:, KO // 2:, 0:TCH])
            xt_tiles.append(xt0_sb)
            nc.sync.dma_start(wh_sb[:, 0], wh_r[:, 0])
            for ko in range(1, KO):
                nc.sync.dma_start(wz_sb[:, ko], wz_r[:, ko])
                nc.sync.dma_start(wh_sb[:, ko], wh_r[:, ko])
            issue_xt(1)

            # Small constants AFTER the critical input stream — their
            # 4-byte-strided descriptors (1024 each) would otherwise
            # steal SDMA descriptor bandwidth from wz0/xt0.
            bz_sb = const_pool.tile([P, MO], fp32)
            nc.sync.dma_start(bz_sb, bz_ext.rearrange("(mo mi) -> mi mo", mi=P))
            bh_sb = const_pool.tile([P, MO], fp32)
            nc.sync.dma_start(bh_sb, bh_ext.rearrange("(mo mi) -> mi mo", mi=P))
            h0_sb = const_pool.tile([P, MO], fp32)
            nc.sync.dma_start(h0_sb, h0_ext.rearrange("(mo mi) -> mi mo", mi=P))

            nbz_sb = const_pool.tile([P, MO], fp32)
            nc.vector.tensor_scalar_mul(nbz_sb, bz_sb, -1.0)
            bhp5_sb = const_pool.tile([P, MO], fp32)
            nc.vector.tensor_scalar_add(bhp5_sb, bh_sb, 0.5)

            # h[0] = g(h_0) = max(h_0 + 0.5, sigmoid(h_0))
            s0_sb = const_pool.tile([P, MO], fp32)
            nc.scalar.activation(s0_sb, h0_sb, SIG)
            gh0_sb = const_pool.tile([P, MO], fp32)
            nc.vector.scalar_tensor_tensor(gh0_sb, h0_sb, 0.5, s0_sb, op0=ADD, op1=MAX)
            nc.scalar.dma_start(
                out_ext[:, 0].rearrange("(mo mi) -> mi mo", mi=P), gh0_sb
            )

            prev_ht = None  # previous chunk's scan output (carries the state)
            prev_tch = TCH

            def gates_scan_store(m, c0, tch, pk, pa, ht_sb, tail=False):
                # pa completes before pk (pa-first matmul order), so s/g
                # overlap the pk matmuls; z -> c -> v -> scan is the tail
                # chain after pk.
                s_sb = gate_pool.tile([P, TCH], fp32, tag="s", name="s")[:, :tch]
                nc.scalar.activation(s_sb, pa, SIG, bias=bh_sb[:, m:m + 1])
                z_sb = gate_pool.tile([P, TCH], fp32, tag="z", name="z")[:, :tch]
                nc.scalar.activation(z_sb, pk, SIG, bias=bz_sb[:, m:m + 1])
                c_sb = gate_pool.tile([P, TCH], fp32, tag="c", name="c")[:, :tch]
                nc.scalar.activation(
                    c_sb, pk, SIG, bias=nbz_sb[:, m:m + 1], scale=-1.0
                )
                g_sb = gate_pool.tile([P, TCH], fp32, tag="g", name="g")[:, :tch]
                nc.vector.scalar_tensor_tensor(
                    g_sb, pa, bhp5_sb[:, m:m + 1], s_sb, op0=ADD, op1=MAX
                )
                v_sb = gate_pool.tile([P, TCH], fp32, tag="v", name="v")[:, :tch]
                # steady state: gpsimd (throughput, keeps DVE free for the
                # scan); kernel tail: DVE (latency)
                if tail:
                    nc.vector.tensor_mul(v_sb, z_sb, g_sb)
                else:
                    nc.gpsimd.tensor_mul(v_sb, z_sb, g_sb)

                init = (
                    gh0_sb[:, m:m + 1]
                    if prev_ht is None
                    else prev_ht[:, m, prev_tch - 1:prev_tch]
                )
                nc.vector.tensor_tensor_scan(
                    ht_sb[:, m, :tch], c_sb, v_sb, init, op0=MUL, op1=ADD
                )
                nc.sync.dma_start(
                    out_ext[m * P:(m + 1) * P, 1 + c0:1 + c0 + tch],
                    ht_sb[:, m, :tch],
                )

            for ci in range(NCH):
                tch = CHUNKS[ci]
                c0 = chunk_t0[ci]
                if ci + 2 < NCH:
                    issue_xt(ci + 2)
                xt_sb = xt_tiles[ci]
                ht_sb = ht_pool.tile([P, MO, TCH], fp32)

                if ci == 0:
                    # Pass A (m 0..GQ-1): k-outer so matmuls consume the
                    # weight k-slices in DMA arrival order — the PE
                    # starts as soon as wz0 + x0 land and never stalls
                    # on the weight stream. GQ=3 leaves a free PSUM slot
                    # pair so pass B starts without waiting on pass A's
                    # gate reads.
                    pks = [
                        psum_p.tile([P, TCH], fp32, tag="pk", name="pk")
                        for _ in range(GQ)
                    ]
                    pas = [
                        psum_p.tile([P, TCH], fp32, tag="pa", name="pa")
                        for _ in range(GQ)
                    ]
                    for ko in range(KO):
                        for q in range(GQ):
                            nc.tensor.matmul(
                                pks[q],
                                wz_sb[:, ko, q * P:(q + 1) * P],
                                xt_sb[:, ko, :tch],
                                start=(ko == 0),
                                stop=(ko == KO - 1),
                            )
                        for q in range(GQ):
                            nc.tensor.matmul(
                                pas[q],
                                wh_sb[:, ko, q * P:(q + 1) * P],
                                xt_sb[:, ko, :tch],
                                start=(ko == 0),
                                stop=(ko == KO - 1),
                            )
                    for q in range(GQ):
                        gates_scan_store(q, c0, tch, pks[q], pas[q], ht_sb)
                    m_range = range(GQ, MO)
                else:
                    m_range = range(MO)

                for m in m_range:
                    pa = psum_p.tile([P, TCH], fp32, tag="pa", name="pa")[:, :tch]
                    pk = psum_p.tile([P, TCH], fp32, tag="pk", name="pk")[:, :tch]
                    for ko in range(KO):
                        nc.tensor.matmul(
                            pa,
                            wh_sb[:, ko, m * P:(m + 1) * P],
                            xt_sb[:, ko, :tch],
                            start=(ko == 0),
                            stop=(ko == KO - 1),
                        )
                    for ko in range(KO):
                        nc.tensor.matmul(
                            pk,
                            wz_sb[:, ko, m * P:(m + 1) * P],
                            xt_sb[:, ko, :tch],
                            start=(ko == 0),
                            stop=(ko == KO - 1),
                        )
                    # DVE mul in the narrow tail chunks: gpsimd's extra
                    # chain latency is what let the scan pipeline lag
                    # ~4 m-tiles behind the PE at kernel end.
                    tail = ci >= NCH - 2
                    gates_scan_store(m, c0, tch, pk, pa, ht_sb, tail=tail)

                prev_ht = ht_sb
                prev_tch = tch

    nc.finalize()
    return nc


def _get_program():
    if "v3" not in _PROGRAM_CACHE:
        _PROGRAM_CACHE["v3"] = _build_program()
    return _PROGRAM_CACHE["v3"]


def run(x, h_0, Wz, bz, Wh, bh, trace=False):
    from concourse.bass_utils import run_bass_kernel_spmd

    nc = _get_program()
    wz16 = np.ascontiguousarray(np.asarray(Wz, dtype=np.float16))
    wh16 = np.ascontiguousarray(np.asarray(Wh, dtype=np.float16))
    bz32 = np.ascontiguousarray(np.asarray(bz, dtype=np.float32))
    bh32 = np.ascontiguousarray(np.asarray(bh, dtype=np.float32))
    in_maps = [
        {
            "x": np.ascontiguousarray(np.asarray(x[b], dtype=np.float16).T),
            "h_0": np.ascontiguousarray(
                np.asarray(h_0[b], dtype=np.float32).reshape(H)
            ),
            "Wz": wz16,
            "bz": bz32,
            "Wh": wh16,
            "bh": bh32,
        }
        for b in range(B)
    ]
    res = run_bass_kernel_spmd(nc, in_maps, list(range(B)), trace=trace)
    out = np.stack(
        [np.ascontiguousarray(res.results[b]["out"].T) for b in range(B)], axis=0
    )
    return out, res


def kernel(x, h_0, Wz, bz, Wh, bh):
    out, _ = run(x, h_0, Wz, bz, Wh, bh)
    return out


# revision 8
# speedup vs baseline: 1.0658x; 1.0141x over previous
"""MinGRU forward on 8 Trainium2 NeuronCores.

Reference computation (per batch b):
    k       = x @ Wz + bz                 # [T, H]
    z       = sigmoid(k)
    c       = 1 - z = sigmoid(-k)
    htilde  = g(x @ Wh + bh)              # g(a) = a+0.5 if a>=0 else sigmoid(a)
                                          #      = max(a+0.5, sigmoid(a))
    h[0]    = g(h_0)
    h[t]    = c[t-1]*h[t-1] + z[t-1]*htilde[t-1]   (t = 1..T)
    out     = h                           # [T+1, H]

The log-space cumlogsumexp in the reference is exactly this linear
recurrence (all quantities positive, coefficients in (0,1), so the
linear form is numerically stable).

Sharding: data-parallel over batch, one batch per core, weights
replicated.

Device layout: matmuls run with H on the PSUM partition dim and T on
the free dim — the layout tensor_tensor_scan needs to run the
recurrence along T at vector speed. x is transposed AND cast to fp16
on the host, so the device only does plain (fast, parallel) DMA loads
— no DMA-transpose, which would serialize against every other DMA via
Tile's deadlock guard. The gate-bias columns and the scan's initial
state g(h_0) are precomputed host-side and shipped as one contiguous
[128, 40] tensor (per-element strided bias loads cost ~1k descriptors
each and starve the SDMA engines at kernel start). The device writes
out[:, 1:] transposed ([H, T]); the host writes column 0 = g(h_0) and
untransposes during the unshard.

Schedule notes (from perfetto trace analysis of previous versions):
  - ~40 junk matmuls at kernel start warm the PE HAM clock gate
    (1.2 -> 2.4 GHz) while the first weight/x DMAs are in flight.
  - Input DMAs split across both HWDGE rings (SP: Wz + x chunks,
    ACT: consts + Wh + chunk-0 x halves) — ring descriptor-issue is
    ~0.7 us per DMA, so one ring would gate the weight stream.
  - Chunk 0 consumes weight k-slices in arrival order: k-outer over
    m 0-2 (GQ=3 leaves a free PSUM slot pair so pass B doesn't wait
    on pass A's gate reads), then m-outer for m 3-7.
  - The last two chunks are 256 wide with the z*htilde multiply on
    DVE instead of gpsimd: shrinks the post-matmul ACT->DVE scan tail
    from ~12 us to ~5 us.
"""

import numpy as np

B, T, D, H = 8, 4096, 1024, 1024
P = 128
TCH = 512                 # main time-chunk (one PSUM bank of fp32)
KO = D // P               # contraction tiles
MO = H // P               # output-channel tiles
CHUNKS = [512] * 7 + [256, 256]   # sum = 4096
NWARM = 40                # HAM warm-up matmuls
GQ = 3                    # chunk-0 pass-A group width (m 0..GQ-1)
NCONST = 5                # bz, -bz, bh, bh+0.5, g(h0) columns

_PROGRAM_CACHE = {}


def _build_program():
    import concourse.bacc as bacc
    import concourse.mybir as mybir
    import concourse.tile as tile

    fp32 = mybir.dt.float32
    fp16 = mybir.dt.float16
    SIG = mybir.ActivationFunctionType.Sigmoid
    MUL = mybir.AluOpType.mult
    ADD = mybir.AluOpType.add
    MAX = mybir.AluOpType.max

    nc = bacc.Bacc("TRN2", target_bir_lowering=False)

    # x arrives pre-transposed from the host: [D, T]
    xt_ext = nc.declare_dram_parameter("x", [D, T], fp16, isOutput=False)
    wz_ext = nc.declare_dram_parameter("Wz", [D, H], fp16, isOutput=False)
    wh_ext = nc.declare_dram_parameter("Wh", [D, H], fp16, isOutput=False)
    # host-packed [P, 5*MO]: bz | -bz | bh | bh+0.5 | g(h0), each [P, MO]
    cn_ext = nc.declare_dram_parameter(
        "consts", [P, NCONST * MO], fp32, isOutput=False
    )
    # transposed output for t = 1..T; host adds the t=0 column
    out_ext = nc.declare_dram_parameter("out", [H, T], fp32, isOutput=True)

    xt_r = xt_ext.rearrange("(ko ki) t -> ki ko t", ki=P)
    wz_r = wz_ext.rearrange("(ko ki) h -> ki ko h", ki=P)
    wh_r = wh_ext.rearrange("(ko ki) h -> ki ko h", ki=P)

    chunk_t0 = []
    t0 = 0
    for w in CHUNKS:
        chunk_t0.append(t0)
        t0 += w
    NCH = len(CHUNKS)

    with tile.TileContext(nc) as tc:
        with (
            tc.tile_pool(name="const", bufs=1) as const_pool,
            tc.tile_pool(name="w", bufs=1) as w_pool,
            tc.tile_pool(name="xt", bufs=3) as xt_pool,
            tc.tile_pool(name="ht", bufs=2) as ht_pool,
            tc.tile_pool(name="gate", bufs=3) as gate_pool,
            tc.tile_pool(name="psp", bufs=4, space="PSUM") as psum_p,
        ):
            # --- PE warm-up: junk matmuls so the HAM clock gate is at
            # 2.4 GHz by the time the first real matmul's operands land.
            junk = const_pool.tile([P, P], fp16)
            nc.gpsimd.memset(junk, 0.0)
            warm_ps = psum_p.tile([P, P], fp32, tag="pk", name="warm")
            for _ in range(NWARM):
                nc.tensor.matmul(warm_ps, junk, junk, start=True, stop=True)

            wz_sb = w_pool.tile([P, KO, H], fp16)
            wh_sb = w_pool.tile([P, KO, H], fp16)
            cn_sb = const_pool.tile([P, NCONST * MO], fp32)
            bz_sb = cn_sb[:, 0 * MO:1 * MO]
            nbz_sb = cn_sb[:, 1 * MO:2 * MO]
            bh_sb = cn_sb[:, 2 * MO:3 * MO]
            bhp5_sb = cn_sb[:, 3 * MO:4 * MO]
            gh0_sb = cn_sb[:, 4 * MO:5 * MO]

            xt_tiles = []

            def issue_xt(ci):
                tch = CHUNKS[ci]
                c0 = chunk_t0[ci]
                xt_sb = xt_pool.tile([P, KO, TCH], fp16, tag="xt", name="xt")
                nc.sync.dma_start(xt_sb[:, :, :tch], xt_r[:, :, c0:c0 + tch])
                xt_tiles.append(xt_sb)

            # --- Head DMAs, split across both rings in consumption
            # order. chunk 0's x goes in two halves so the first matmul
            # (ko=0) only waits for 512 KB.
            xt0_sb = xt_pool.tile([P, KO, TCH], fp16, tag="xt", name="xt")
            xt_tiles.append(xt0_sb)

            # ACT ring: consts, x0 lo, wh0, x0 hi, wh 1..7
            nc.scalar.dma_start(cn_sb, cn_ext[:, :])
            nc.scalar.dma_start(xt0_sb[:, :KO // 2], xt_r[:, :KO // 2, 0:TCH])
            nc.scalar.dma_start(wh_sb[:, 0], wh_r[:, 0])
            nc.scalar.dma_start(xt0_sb[:, KO // 2:], xt_r[:, KO // 2:, 0:TCH])
            for ko in range(1, KO):
                nc.scalar.dma_start(wh_sb[:, ko], wh_r[:, ko])
            # SP ring: wz 0..7, then x chunk 1
            for ko in range(KO):
                nc.sync.dma_start(wz_sb[:, ko], wz_r[:, ko])
            issue_xt(1)

            prev_ht = None  # previous chunk's scan output (carries the state)
            prev_tch = TCH

            def gates_scan_store(m, c0, tch, pk, pa, ht_sb, tail=False):
                # pa completes before pk (pa-first matmul order), so s/g
                # overlap the pk matmuls; z -> c -> v -> scan is the tail
                # chain after pk.
                s_sb = gate_pool.tile([P, TCH], fp32, tag="s", name="s")[:, :tch]
                nc.scalar.activation(s_sb, pa, SIG, bias=bh_sb[:, m:m + 1])
                z_sb = gate_pool.tile([P, TCH], fp32, tag="z", name="z")[:, :tch]
                nc.scalar.activation(z_sb, pk, SIG, bias=bz_sb[:, m:m + 1])
                c_sb = gate_pool.tile([P, TCH], fp32, tag="c", name="c")[:, :tch]
                nc.scalar.activation(
                    c_sb, pk, SIG, bias=nbz_sb[:, m:m + 1], scale=-1.0
                )
                g_sb = gate_pool.tile([P, TCH], fp32, tag="g", name="g")[:, :tch]
                nc.vector.scalar_tensor_tensor(
                    g_sb, pa, bhp5_sb[:, m:m + 1], s_sb, op0=ADD, op1=MAX
                )
                v_sb = gate_pool.tile([P, TCH], fp32, tag="v", name="v")[:, :tch]
                # steady state: gpsimd (throughput, keeps DVE free for the
                # scan); kernel tail: DVE (latency)
                if tail:
                    nc.vector.tensor_mul(v_sb, z_sb, g_sb)
                else:
                    nc.gpsimd.tensor_mul(v_sb, z_sb, g_sb)

                init = (
                    gh0_sb[:, m:m + 1]
                    if prev_ht is None
                    else prev_ht[:, m, prev_tch - 1:prev_tch]
                )
                nc.vector.tensor_tensor_scan(
                    ht_sb[:, m, :tch], c_sb, v_sb, init, op0=MUL, op1=ADD
                )
                nc.sync.dma_start(
                    out_ext[m * P:(m + 1) * P, c0:c0 + tch],
                    ht_sb[:, m, :tch],
                )

            for ci in range(NCH):
                tch = CHUNKS[ci]
                c0 = chunk_t0[ci]
                if ci + 2 < NCH:
                    issue_xt(ci + 2)
                xt_sb = xt_tiles[ci]
                ht_sb = ht_pool.tile([P, MO, TCH], fp32)

                if ci == 0:
                    # Pass A (m 0..GQ-1): k-outer so matmuls consume the
                    # weight k-slices in DMA arrival order — the PE
                    # starts as soon as wz0 + x0 land and never stalls
                    # on the weight stream. GQ=3 leaves a free PSUM slot
                    # pair so pass B starts without waiting on pass A's
                    # gate reads.
                    pks = [
                        psum_p.tile([P, TCH], fp32, tag="pk", name="pk")
                        for _ in range(GQ)
                    ]
                    pas = [
                        psum_p.tile([P, TCH], fp32, tag="pa", name="pa")
                        for _ in range(GQ)
                    ]
                    for ko in range(KO):
                        for q in range(GQ):
                            nc.tensor.matmul(
                                pks[q],
                                wz_sb[:, ko, q * P:(q + 1) * P],
                                xt_sb[:, ko, :tch],
                                start=(ko == 0),
                                stop=(ko == KO - 1),
                            )
                        for q in range(GQ):
                            nc.tensor.matmul(
                                pas[q],
                                wh_sb[:, ko, q * P:(q + 1) * P],
                                xt_sb[:, ko, :tch],
                                start=(ko == 0),
                                stop=(ko == KO - 1),
                            )
                    for q in range(GQ):
                        gates_scan_store(q, c0, tch, pks[q], pas[q], ht_sb)
                    m_range = range(GQ, MO)
                else:
                    m_range = range(MO)

                for m in m_range:
                    pa = psum_p.tile([P, TCH], fp32, tag="pa", name="pa")[:, :tch]
                    pk = psum_p.tile([P, TCH], fp32, tag="pk", name="pk")[:, :tch]
                    for ko in range(KO):
                        nc.tensor.matmul(
                            pa,
                            wh_sb[:, ko, m * P:(m + 1) * P],
                            xt_sb[:, ko, :tch],
                            start=(ko == 0),
                            stop=(ko == KO - 1),
                        )
                    for ko in range(KO):
                        nc.tensor.matmul(
                            pk,
                            wz_sb[:, ko, m * P:(m + 1) * P],
                            xt_sb[:, ko, :tch],
                            start=(ko == 0),
                            stop=(ko == KO - 1),
                        )
                    # DVE mul in the narrow tail chunks: gpsimd's extra
                    # chain latency is what let the scan pipeline lag
                    # ~4 m-tiles behind the PE at kernel end.
                    tail = ci >= NCH - 2
                    gates_scan_store(m, c0, tch, pk, pa, ht_sb, tail=tail)

                prev_ht = ht_sb
                prev_tch = tch

    nc.finalize()
    return nc


def _get_program():
    if "v5" not in _PROGRAM_CACHE:
        _PROGRAM_CACHE["v5"] = _build_program()
    return _PROGRAM_CACHE["v5"]


def _g(x):
    return np.maximum(x + 0.5, 1.0 / (1.0 + np.exp(-x)))


def run(x, h_0, Wz, bz, Wh, bh, trace=False):
    from concourse.bass_utils import run_bass_kernel_spmd

    nc = _get_program()
    wz16 = np.ascontiguousarray(np.asarray(Wz, dtype=np.float16))
    wh16 = np.ascontiguousarray(np.asarray(Wh, dtype=np.float16))
    bz32 = np.asarray(bz, dtype=np.float32)
    bh32 = np.asarray(bh, dtype=np.float32)
    gh0 = _g(np.asarray(h_0, dtype=np.float32).reshape(B, H))  # [B, H]

    def col(v):  # [H] -> [P, MO] with partition = channel-within-tile
        return v.reshape(MO, P).T

    cn_common = [col(bz32), col(-bz32), col(bh32), col(bh32 + 0.5)]
    in_maps = [
        {
            "x": np.ascontiguousarray(np.asarray(x[b], dtype=np.float16).T),
            "Wz": wz16,
            "Wh": wh16,
            "consts": np.ascontiguousarray(
                np.concatenate(cn_common + [col(gh0[b])], axis=1),
                dtype=np.float32,
            ),
        }
        for b in range(B)
    ]
    res = run_bass_kernel_spmd(nc, in_maps, list(range(B)), trace=trace)
    out = np.empty((B, T + 1, H), dtype=np.float32)
    out[:, 0, :] = gh0
    for b in range(B):
        out[b, 1:, :] = res.results[b]["out"].T
    return out, res


def kernel(x, h_0, Wz, bz, Wh, bh):
    out, _ = run(x, h_0, Wz, bz, Wh, bh)
    return out
